# revision 1
# baseline (speedup 1.0000x reference)
"""Trainium2 Bass kernel for nn_MeshUpConv (3-layer spline-conv GNN).

Strategy (8 NeuronCores, SPMD, one NEFF, 3 launches = 1 per layer):
- Nodes sharded by range (6250/core); edges sharded by destination.
- Host relabels each core's destination nodes so the edge stream packs into
  a uniform static tile structure: T tiles of 128 edge slots, tile t owning
  relabeled-id window [16t, 16t+32), 32 tiles per 512-id PSUM group. The
  same instruction stream then works for every core (SPMD) with per-core
  data (gather indices, one-hot A tiles, spline-basis B tiles).
- Per tile on device:
    xjT streamed from HBM (host pre-gathered fp16 hi|lo edge stream)
    P   = xjT.T @ Wsplit            (PE, fp16 -> fp32 PSUM, 288 = 9k x 32c)
    Pb  = fp16(P)                   (ScalarE copy)
    T'  = Pb * B[e,k] broadcast     (VectorE tensor_tensor)
    agg[:, win] += T'_k.T @ A_tile  (PE, 9 accumulating matmuls)
  plus per group: agg = root_aug.T @ x_ownT_aug (start=True) and a ReLU
  flush to an SBUF staging buffer, written out once per layer.
- Between layers the host rebuilds the (small) per-core tables from the
  gathered layer output: the halo exchange is done host-side since the
  full-input/full-output contract already round-trips through the host.
"""
import os
import sys
import numpy as np

sys.path.insert(0, "/opt/trn_rl_repo")

N_NODES = 50000
N_EDGES = 400000
C_IN = 64
C_OUT = 32
K = 9
N_CORES = 8
NODES_PER_CORE = N_NODES // N_CORES
TILE_E = 128
IDS_PER_TILE = 16
WIN = 32
TILES_PER_GROUP = 32
GROUP = IDS_PER_TILE * TILES_PER_GROUP  # 512
TABLE_ROWS = 32768
CHUNK = 16  # tiles per gather/stream chunk


# ----------------------------------------------------------------------------
# Host preprocessing
# ----------------------------------------------------------------------------

def spline_basis_np(pseudo):
    def quad(t):
        return np.stack([0.5 * (1.0 - t) ** 2, -t * t + t + 0.5, 0.5 * t * t],
                        axis=-1)
    q0 = quad(pseudo[:, 0])
    q1 = quad(pseudo[:, 1])
    return (q1[:, :, None] * q0[:, None, :]).reshape(-1, K)


def _pack_core(e_src, e_dst_local, e_b):
    n_nodes = NODES_PER_CORE
    order = np.argsort(e_dst_local, kind="stable")
    e_src = e_src[order]
    e_dst = e_dst_local[order]
    e_b = e_b[order]
    starts = np.searchsorted(e_dst, np.arange(n_nodes), side="left")
    ends = np.searchsorted(e_dst, np.arange(n_nodes), side="right")

    tiles_src, tiles_off, tiles_b = [], [], []
    perm = np.full(n_nodes, -1, dtype=np.int64)
    t = 0
    ids_used = 0
    next_ids_used = 0
    cur_src, cur_off, cur_b = [], [], []

    def close_tile():
        nonlocal t, ids_used, cur_src, cur_off, cur_b, next_ids_used
        tiles_src.append(cur_src)
        tiles_off.append(cur_off)
        tiles_b.append(cur_b)
        cur_src, cur_off, cur_b = [], [], []
        t += 1
        ids_used = next_ids_used
        next_ids_used = 0

    for v in range(n_nodes):
        d = ends[v] - starts[v]
        sv = e_src[starts[v]:ends[v]]
        bv = e_b[starts[v]:ends[v]]
        placed = False
        while not placed:
            cap = TILE_E - len(cur_src)
            if ids_used >= IDS_PER_TILE or cap == 0:
                close_tile()
                continue
            if d <= cap:
                node_id = IDS_PER_TILE * t + ids_used
                perm[v] = node_id
                cur_src.extend(sv.tolist())
                cur_off.extend([ids_used] * d)
                cur_b.extend(list(bv))
                ids_used += 1
                placed = True
            else:
                last_in_group = (t + 1) % TILES_PER_GROUP == 0
                if last_in_group or next_ids_used >= IDS_PER_TILE:
                    close_tile()
                    continue
                node_id = IDS_PER_TILE * (t + 1) + next_ids_used
                next_ids_used += 1
                perm[v] = node_id
                off_in_t = node_id - IDS_PER_TILE * t  # in [16, 32)
                cur_src.extend(sv[:cap].tolist())
                cur_off.extend([off_in_t] * cap)
                cur_b.extend(list(bv[:cap]))
                rem = d - cap
                close_tile()
                cur_src.extend(sv[cap:].tolist())
                cur_off.extend([0 + (node_id - IDS_PER_TILE * t)] * rem)
                cur_b.extend(list(bv[cap:]))
                placed = True
    if cur_src or ids_used:
        close_tile()
    return tiles_src, tiles_off, tiles_b, perm


def build_static(edge_index, pseudo):
    src = np.asarray(edge_index[0], dtype=np.int64)
    dst = np.asarray(edge_index[1], dtype=np.int64)
    B = spline_basis_np(np.asarray(pseudo, dtype=np.float32)).astype(np.float32)

    raw = []
    for c in range(N_CORES):
        lo_n = c * NODES_PER_CORE
        sel = np.nonzero((dst >= lo_n) & (dst < lo_n + NODES_PER_CORE))[0]
        raw.append(_pack_core(src[sel], dst[sel] - lo_n, B[sel]) + (lo_n,))

    t_max = max(len(r[0]) for r in raw)
    T = ((t_max + TILES_PER_GROUP - 1) // TILES_PER_GROUP) * TILES_PER_GROUP
    # chunking requires T % CHUNK == 0; TILES_PER_GROUP=32 is a multiple of 16
    ids_total = T * IDS_PER_TILE

    cores = []
    for ts, to, tb, perm, lo_n in raw:
        idx = np.zeros((T, TILE_E), dtype=np.int64)
        a_t = np.zeros((T, TILE_E, WIN), dtype=np.float16)
        b_t = np.zeros((T, TILE_E, 16), dtype=np.float16)
        for t in range(len(ts)):
            n = len(ts[t])
            if n == 0:
                continue
            idx[t, :n] = ts[t]
            offs = np.asarray(to[t])
            tt = t % TILES_PER_GROUP
            shift = IDS_PER_TILE * tt - min(IDS_PER_TILE * tt, GROUP - WIN)
            a_t[t, np.arange(n), offs + shift] = 1.0
            b_t[t, :n, :9] = np.asarray(tb[t], dtype=np.float16)
        # device-friendly partition-major layouts
        a_pm = np.ascontiguousarray(a_t.transpose(1, 0, 2).reshape(TILE_E, T * WIN))
        b_pm = np.ascontiguousarray(b_t.transpose(1, 0, 2).reshape(TILE_E, T * 16))
        cores.append(dict(slots=idx.reshape(-1), a_pm=a_pm.astype(np.float16),
                          b_pm=b_pm.astype(np.float16), perm=perm, node_lo=lo_n))
    return cores, T, ids_total


def _wrap_idx(idx_flat):
    n = len(idx_flat)
    assert n % 16 == 0
    buf = idx_flat.reshape(n // 16, 16).T.copy()
    return np.tile(buf, (8, 1)).astype(np.int16)


def build_layer_core_inputs(x_full, cc, ids_total):
    C = x_full.shape[1]
    rows = x_full[cc["slots"]].astype(np.float32)   # [T*128, C] host gather
    hi = rows.astype(np.float16)
    lo = (rows - hi.astype(np.float32)).astype(np.float16)
    S = rows.shape[0]
    xjt = np.zeros((128, S), dtype=np.float16)
    xjt[:C] = hi.T
    xjt[64:64 + C] = lo.T
    own = x_full[cc["node_lo"]:cc["node_lo"] + NODES_PER_CORE]
    ownT = np.zeros((65, ids_total), dtype=np.float32)
    valid = cc["perm"] >= 0
    ownT[:C, cc["perm"][valid]] = own[valid].T.astype(np.float32)
    ownT[64, cc["perm"][valid]] = 1.0
    return xjt, ownT


def build_weights(W, root, b):
    C = W.shape[1]
    Wk = W.transpose(1, 0, 2).reshape(C, K * C_OUT)  # columns k-outer (k,c)
    Wh = np.zeros((128, K * C_OUT), dtype=np.float16)
    Wh[:C] = Wk.astype(np.float16)
    Wh[64:64 + C] = Wk.astype(np.float16)
    root_aug = np.zeros((65, C_OUT), dtype=np.float32)
    root_aug[:C] = root
    root_aug[64] = b
    return Wh, root_aug


def unpermute(outT_list, cores):
    res = np.zeros((N_NODES, C_OUT), dtype=np.float32)
    for cc, o in zip(cores, outT_list):
        valid = cc["perm"] >= 0
        res[cc["node_lo"] + np.nonzero(valid)[0]] = o[:, cc["perm"][valid]].T
    return res


# ----------------------------------------------------------------------------
# Bass kernel
# ----------------------------------------------------------------------------

def build_bass_kernel(T, ids_total):
    import concourse.bass as bass
    import concourse.bacc as bacc
    import concourse.mybir as mybir
    import concourse.tile as tile

    fp16 = mybir.dt.float16
    fp32 = mybir.dt.float32
    i16 = mybir.dt.int16
    n_groups = T // TILES_PER_GROUP
    n_chunks = T // CHUNK

    nc = bacc.Bacc("TRN2", target_bir_lowering=False, debug=False,
                   enable_asserts=False, num_devices=N_CORES)

    xjt_d = nc.dram_tensor("xjt", [128, T * TILE_E], fp16, kind="ExternalInput")
    a_pm = nc.dram_tensor("a_pm", [128, T * WIN], fp16, kind="ExternalInput")
    b_pm = nc.dram_tensor("b_pm", [128, T * 16], fp16, kind="ExternalInput")
    w_split = nc.dram_tensor("w_split", [128, K * C_OUT], fp16, kind="ExternalInput")
    root_aug = nc.dram_tensor("root_aug", [65, C_OUT], fp32, kind="ExternalInput")
    ownt = nc.dram_tensor("ownt", [65, ids_total], fp32, kind="ExternalInput")
    outt = nc.dram_tensor("outt", [C_OUT, ids_total], fp32, kind="ExternalOutput")

    with tile.TileContext(nc) as tc:
        with (
            tc.tile_pool(name="const", bufs=1) as cpool,
            tc.tile_pool(name="gath", bufs=3) as gpool,
            tc.tile_pool(name="astr", bufs=3) as apool,
            tc.tile_pool(name="bstr", bufs=3) as bpool,
            tc.tile_pool(name="pb", bufs=6) as pbpool,
            tc.tile_pool(name="tp", bufs=6) as tppool,
            tc.tile_pool(name="own", bufs=2) as opool,
            tc.tile_pool(name="ost", bufs=1) as spool,
            tc.tile_pool(name="psP", bufs=5, space="PSUM") as psP,
            tc.tile_pool(name="psA", bufs=2, space="PSUM") as psA,
        ):
            wsp = cpool.tile([128, K * C_OUT], fp16)
            nc.sync.dma_start(out=wsp[:], in_=w_split[:, :])
            rta = cpool.tile([65, C_OUT], fp32)
            nc.sync.dma_start(out=rta[:], in_=root_aug[:, :])
            outstage = spool.tile([C_OUT, ids_total], fp32)

            for g in range(n_groups):
                ownt_t = opool.tile([65, GROUP], fp32)
                nc.sync.dma_start(out=ownt_t[:],
                                  in_=ownt[:, g * GROUP:(g + 1) * GROUP])
                agg = psA.tile([C_OUT, GROUP], fp32)
                nc.tensor.matmul(agg[:], lhsT=rta[:], rhs=ownt_t[:],
                                 start=True, stop=False, skip_group_check=True)
                for tt in range(TILES_PER_GROUP):
                    t = g * TILES_PER_GROUP + tt
                    if t % CHUNK == 0:
                        gt = gpool.tile([128, 1, CHUNK * TILE_E], fp16)
                        nc.sync.dma_start(
                            out=gt[:, 0, :],
                            in_=xjt_d[:, t * TILE_E:(t + CHUNK) * TILE_E])
                        at = apool.tile([128, CHUNK * WIN], fp16)
                        nc.sync.dma_start(
                            out=at[:], in_=a_pm[:, t * WIN:(t + CHUNK) * WIN])
                        bt = bpool.tile([128, CHUNK * 16], fp16)
                        nc.sync.dma_start(
                            out=bt[:], in_=b_pm[:, t * 16:(t + CHUNK) * 16])
                    j = t % CHUNK
                    P = psP.tile([128, K * C_OUT], fp32)
                    nc.tensor.matmul(
                        P[:], lhsT=gt[:, 0, j * TILE_E:(j + 1) * TILE_E],
                        rhs=wsp[:], start=True, stop=True,
                        skip_group_check=True)
                    Tp = tppool.tile([128, K * C_OUT], fp16)
                    b_bcast = bt[:, j * 16:j * 16 + 9].unsqueeze(2) \
                        .to_broadcast([128, 9, C_OUT])
                    nc.vector.scalar_tensor_tensor(
                        out=Tp[:].rearrange("p (k c) -> p k c", k=K),
                        in0=P[:].rearrange("p (k c) -> p k c", k=K),
                        scalar=0.0,
                        in1=b_bcast,
                        op0=mybir.AluOpType.add,
                        op1=mybir.AluOpType.mult)
                    base = min(IDS_PER_TILE * tt, GROUP - WIN)
                    for k in range(K):
                        nc.tensor.matmul(
                            agg[:, base:base + WIN],
                            lhsT=Tp[:, k * C_OUT:(k + 1) * C_OUT],
                            rhs=at[:, j * WIN:(j + 1) * WIN],
                            start=False,
                            stop=(tt == TILES_PER_GROUP - 1 and k == K - 1),
                            skip_group_check=True)
                nc.scalar.activation(
                    outstage[:, g * GROUP:(g + 1) * GROUP], agg[:],
                    mybir.ActivationFunctionType.Relu)
            nc.sync.dma_start(out=outt[:, :], in_=outstage[:])
    nc.compile()
    return nc


# ----------------------------------------------------------------------------
# PJRT runner with cached executable (3 launches, 1 compile)
# ----------------------------------------------------------------------------

class PjrtRunner:
    def __init__(self, nc, n_cores):
        import jax
        import numpy as _np
        from jax.sharding import Mesh, PartitionSpec
        from jax.experimental.shard_map import shard_map
        from concourse import bass2jax as b2j
        import concourse.mybir as mybir

        b2j.install_neuronx_cc_hook()
        self.nc = nc
        self.n_cores = n_cores
        partition_name = (nc.partition_id_tensor.name
                          if nc.partition_id_tensor else None)
        in_names, out_names, out_avals, zero_outs = [], [], [], []
        for alloc in nc.m.functions[0].allocations:
            if not isinstance(alloc, mybir.MemoryLocationSet):
                continue
            name = alloc.memorylocations[0].name
            if alloc.kind == "ExternalInput":
                if name != partition_name:
                    in_names.append(name)
            elif alloc.kind == "ExternalOutput":
                out_names.append(name)
                shape = tuple(alloc.tensor_shape)
                dtype = mybir.dt.np(alloc.dtype)
                out_avals.append(jax.core.ShapedArray(shape, dtype))
                zero_outs.append(_np.zeros(shape, dtype))
        self.in_names = list(in_names)
        self.out_names = out_names
        self.zero_outs = zero_outs
        self.out_avals = out_avals
        n_params = len(in_names)
        n_outs = len(out_avals)
        all_in_names = in_names + out_names + (
            [partition_name] if partition_name else [])
        donate = tuple(range(n_params, n_params + n_outs))

        def _body(*args):
            operands = list(args)
            if partition_name is not None:
                operands.append(b2j.partition_id_tensor())
            outs = b2j._bass_exec_p.bind(
                *operands,
                out_avals=tuple(out_avals),
                in_names=tuple(all_in_names),
                out_names=tuple(out_names),
                lowering_input_output_aliases=(),
                sim_require_finite=True,
                sim_require_nnan=True,
                nc=nc,
            )
            return tuple(outs)

        devices = jax.devices()[:n_cores]
        mesh = Mesh(np.asarray(devices), ("core",))
        in_specs = (PartitionSpec("core"),) * (n_params + n_outs)
        out_specs = (PartitionSpec("core"),) * len(out_names)
        self.fn = jax.jit(
            shard_map(_body, mesh=mesh, in_specs=in_specs,
                      out_specs=out_specs, check_rep=False),
            donate_argnums=donate, keep_unused=True)

    def __call__(self, in_maps):
        per_core = [[np.asarray(m[name]) for name in self.in_names]
                    for m in in_maps]
        concat_in = [np.concatenate([per_core[c][i]
                                     for c in range(self.n_cores)], axis=0)
                     for i in range(len(self.in_names))]
        concat_zeros = [np.zeros((self.n_cores * z.shape[0], *z.shape[1:]),
                                 z.dtype) for z in self.zero_outs]
        out_arrs = self.fn(*concat_in, *concat_zeros)
        return [
            {name: np.asarray(out_arrs[i]).reshape(
                self.n_cores, *self.out_avals[i].shape)[c]
             for i, name in enumerate(self.out_names)}
            for c in range(self.n_cores)
        ]


_CACHE = {}


def _get_runner(T, ids_total):
    key = (T, ids_total)
    if key not in _CACHE:
        nc = build_bass_kernel(T, ids_total)
        _CACHE[key] = PjrtRunner(nc, N_CORES)
    return _CACHE[key]


def _run_layer(runner, cores, x_full, Wsplit, root_aug_np, ids_total):
    in_maps = []
    for cc in cores:
        xjt, ownT = build_layer_core_inputs(x_full, cc, ids_total)
        in_maps.append({
            "xjt": xjt, "a_pm": cc["a_pm"],
            "b_pm": cc["b_pm"], "w_split": Wsplit, "root_aug": root_aug_np,
            "ownt": ownT,
        })
    res = runner(in_maps)
    return unpermute([r["outt"] for r in res], cores)


def kernel(**inputs) -> np.ndarray:
    x = np.asarray(inputs["x"], np.float32)
    skip = np.asarray(inputs["skip"], np.float32)
    cores, T, ids_total = build_static(np.asarray(inputs["edge_index"]),
                                       np.asarray(inputs["pseudo"]))
    W1s, root1a = build_weights(np.asarray(inputs["W1"], np.float32),
                                np.asarray(inputs["root1"], np.float32),
                                np.asarray(inputs["b1"], np.float32))
    W2s, root2a = build_weights(np.asarray(inputs["W2"], np.float32),
                                np.asarray(inputs["root2"], np.float32),
                                np.asarray(inputs["b2"], np.float32))
    runner = _get_runner(T, ids_total)
    h = _run_layer(runner, cores, x, W1s, root1a, ids_total)
    h2 = np.concatenate([h, skip], axis=1)
    h = _run_layer(runner, cores, h2, W1s, root1a, ids_total)
    h = _run_layer(runner, cores, h, W2s, root2a, ids_total)
    return h

